# revision 14
# baseline (speedup 1.0000x reference)
"""Trainium2 Bass kernel for nn_CRA_46797963657479.

Math: the reference builds per-batch gram matrix A = cat_phi^T cat_phi and
feeds concat(A, A^T) through big 1x1 convs.  A is symmetric and the tail is
linear, so it collapses to

    W[b, l] = (u3 + cat_phi[b] @ u4) . cat_phi[b][:, l] + K
    out[b]  = xp[b] * W[b, :N] + yp[b] * W[b, N:]

with u3 = w5a @ w3, u4 = w5b @ (w4[:, :2N] + w4[:, 2N:]),
K = w5a.b3 + w5b.b4 + b5; BN folds into the conv weights.

v3 design (per pair of batches), measured-rate driven:
  phi PSUM is 3 full-height banks per pair: bank0 = x-stream out-chans
  0:128, bank1 = y-stream out-chans 0:128, bank2 = x-hi (parts 0:64) ||
  y-hi (parts 64:128).  One merged relu activation (scalar) covers all
  three banks.  z = phi @ u4 is one 2x-mode DVE TT multiply into junk
  plus two 1x reduces (reduce never runs 2x on HW).  The +K constant
  rides the W PSUM->SBUF copy as a per-partition bias AP (free on the
  scalar engine).  The combine is split DVE/gpsimd by measured rates
  (DVE 0.61 ns/col vs gpsimd 2.08).  PE fillers keep the chip-wide
  DVFS p-state up (narrow 120-col bursts into unused psw columns).
Sharding: pure data parallel, batch 256 -> 32 per core on 8 cores.
"""

import ml_dtypes
import numpy as np

import concourse.bass as bass
import concourse.bacc as bacc
import concourse.tile as tile
from concourse import mybir
from concourse.bass_utils import run_bass_kernel_spmd

F32 = mybir.dt.float32
BF16 = mybir.dt.bfloat16

B, N, C = 256, 196, 192
NCORES = 8
NB = B // NCORES          # 32 batches per core
NPAIR = NB // 2           # 16 pairs per core
L = 2 * N                 # 392
CLO, CHI = 128, C - 128   # 128 + 64 channel split
CHIA = CHI + 1            # hi contraction chunk + ones row
GP = 4                    # pairs per I/O group (8 batches)
NGRP = NPAIR // GP

_CACHE = {}


def _reap(ap, dims):
    """Rebuild an AP with explicit free-dim [stride, n] list."""
    return bass.AP(tensor=ap.tensor, offset=ap.offset, ap=[ap.ap[0]] + dims)


def _build_program():
    nc = bacc.Bacc("TRN2", target_bir_lowering=False, debug=False)

    # HBM layouts are channel-major; xyb row 192 is constant 1.0 (the
    # ones-row that folds the conv bias into the hi-contraction matmul).
    xyb = nc.dram_tensor("xyb", [C + 1, NB, 2, N], BF16, kind="ExternalInput")
    outt = nc.dram_tensor("out", [C, NB, N], BF16, kind="ExternalOutput")
    wblob = nc.dram_tensor("wblob", [CLO, 768], BF16, kind="ExternalInput")
    u4bT = nc.dram_tensor("u4b", [CLO, 588], BF16, kind="ExternalInput")
    u34T = nc.dram_tensor("u34", [CLO, 5], F32, kind="ExternalInput")
    foldT = nc.dram_tensor("foldm", [CLO, CLO], F32, kind="ExternalInput")

    relu = mybir.ActivationFunctionType.Relu
    ident = mybir.ActivationFunctionType.Identity
    mult = mybir.AluOpType.mult
    add = mybir.AluOpType.add

    with tile.TileContext(nc) as tc:
        with (
            tc.tile_pool(name="consts", bufs=1) as consts,
            tc.tile_pool(name="xin", bufs=2) as xin,
            tc.tile_pool(name="phip", bufs=4) as phip,
            tc.tile_pool(name="junkp", bufs=3) as junkp,
            tc.tile_pool(name="qp", bufs=5) as qp,
            tc.tile_pool(name="wsbp", bufs=4) as wsbp,
            tc.tile_pool(name="work", bufs=4) as work,
            tc.tile_pool(name="outp", bufs=2) as outp,
            tc.tile_pool(name="psph", bufs=2, space="PSUM") as psph,
            tc.tile_pool(name="psw", bufs=1, space="PSUM") as psw,
        ):
            wb = consts.tile([CLO, 768], BF16)
            nc.sync.dma_start(out=wb[:], in_=wblob[:])
            u4t = consts.tile([CLO, 588], BF16)
            nc.sync.dma_start(out=u4t[:], in_=u4bT[:])
            u3t = consts.tile([CLO, 5], F32)
            nc.sync.dma_start(out=u3t[:], in_=u34T[:])
            foldm = consts.tile([CLO, CLO], F32)
            nc.sync.dma_start(out=foldm[:], in_=foldT[:])

            twxA = wb[:, 0:128]
            twxB = wb[:, 128:192]
            twxC = wb[0:CHIA, 192:320]
            twxD = wb[0:CHIA, 320:384]
            twyA = wb[:, 384:512]
            twyB = wb[:, 512:576]
            twyC = wb[0:CHIA, 576:704]
            twyD = wb[0:CHIA, 704:768]
            kv = u3t[:, 4:5]

            # PE HAM warm-up: dense matmuls so the clock promotes to 2.4GHz
            # before the first real matmul.
            wseed = consts.tile([CLO, 640], BF16)
            nc.vector.memset(wseed[:], 1.0)
            wup = psw.tile([CLO, 2, 512], F32, tag="ps_w")
            for _ in range(24):
                nc.tensor.matmul(wup[:, 0, :], wseed[:, 512:640],
                                 wseed[:, 0:512], start=True, stop=True)

            def emit_front(p, xgb, og):
                u = p % GP
                b0 = 2 * u
                x_lo = xgb[:, 0, b0:b0 + 2, 0, :]
                x_hi = xgb[0:CHIA, 1, b0:b0 + 2, 0, :]
                y_lo = xgb[:, 0, b0:b0 + 2, 1, :]
                y_hi = xgb[0:CHIA, 1, b0:b0 + 2, 1, :]

                ps = psph.tile([CLO, 3, 512], F32, tag="phi")
                nc.tensor.matmul(ps[:, 0, 0:L], twxA, x_lo, start=True, stop=False)
                nc.tensor.matmul(ps[:, 0, 0:L], twxC, x_hi, start=False, stop=True)
                nc.tensor.matmul(ps[:, 1, 0:L], twyA, y_lo, start=True, stop=False)
                nc.tensor.matmul(ps[:, 1, 0:L], twyC, y_hi, start=False, stop=True)
                nc.tensor.matmul(ps[0:CHI, 2, 0:L], twxB, x_lo, start=True, stop=False)
                nc.tensor.matmul(ps[0:CHI, 2, 0:L], twxD, x_hi, start=False, stop=True)
                nc.tensor.matmul(ps[CHI:CLO, 2, 0:L], twyB, y_lo, start=True, stop=False)
                nc.tensor.matmul(ps[CHI:CLO, 2, 0:L], twyD, y_hi, start=False, stop=True)

                # phi [p, bank, b, n]; one merged act over all 3 banks.
                phi = phip.tile([CLO, 3, 2, N], BF16, tag="phi")
                nc.scalar.activation(
                    _reap(phi[:], [[1, 3 * L]]),
                    _reap(ps[:], [[512, 3], [1, L]]), relu)

                # z[c] = sum_l phi[c,l]*u4[l]: one 2x TT mult (junk out,
                # iterated (b, bank, n)) + two 1x reduces.
                junk = junkp.tile([CLO, 2, 3, N], BF16, tag="junk")
                nc.vector.tensor_tensor(
                    junk[:],
                    _reap(phi[:], [[N, 2], [2 * N, 3], [1, N]]),
                    _reap(u4t[:], [[0, 2], [N, 3], [1, N]]), mult)
                z4 = qp.tile([CLO, 4], F32, tag="z4")
                nc.vector.tensor_reduce(
                    z4[:, 0:2], junk[:, :, 0:2, :], mybir.AxisListType.XY, add)
                nc.vector.tensor_reduce(
                    z4[:, 2:4], junk[:, :, 2, :], mybir.AxisListType.X, add)

                # q = u3 + z.  qb cols 0,1 = q-lo (b0,b1); cols 2,3 = q-hi
                # on parts 0:64 (x-half lhsT); cols 4,5 = q-hi on parts
                # 64:128 (y-half lhsT).  z-hi's x and y halves live 64
                # partitions apart; a tiny PE matmul with the constant
                # fold matrix F = I + shift64 sums them on every partition
                # (zf lands in unused columns of the phi PSUM tile).
                zf = ps[:, 0, 400:402]
                nc.tensor.matmul(zf, foldm[:], z4[:, 2:4], start=True, stop=True)
                qb = qp.tile([CLO, 6], BF16, tag="qb")
                nc.gpsimd.tensor_tensor(qb[:, 0:2], z4[:, 0:2], u3t[:, 0:2], add)
                nc.vector.tensor_scalar_add(qb[0:CHI, 2:4], zf[0:CHI, :],
                                            u3t[0:CHI, 2:3])
                nc.vector.tensor_scalar_add(qb[CHI:CLO, 4:6], zf[CHI:CLO, :],
                                            u3t[CHI:CLO, 3:4])
                return dict(p=p, xgb=xgb, og=og, phi=phi, qb=qb)

            def emit_back(st):
                p, xgb, og, phi, qb = st["p"], st["xgb"], st["og"], st["phi"], st["qb"]
                u = p % GP
                b0 = 2 * u
                ps_w = psw.tile([CLO, 2, 512], F32, tag="ps_w")
                for b in (0, 1):
                    nc.tensor.matmul(ps_w[:, b, 0:N],
                                     qb[:, b:b + 1].broadcast_to([CLO, CLO]),
                                     phi[:, 0, b, :], start=True, stop=False)
                    nc.tensor.matmul(ps_w[:, b, 0:N],
                                     qb[0:CHI, 2 + b:3 + b].broadcast_to([CHI, CLO]),
                                     phi[0:CHI, 2, b, :], start=False, stop=True)
                    nc.tensor.matmul(ps_w[:, b, N:L],
                                     qb[:, b:b + 1].broadcast_to([CLO, CLO]),
                                     phi[:, 1, b, :], start=True, stop=False)
                    nc.tensor.matmul(ps_w[:, b, N:L],
                                     qb[CHI:CLO, 4 + b:5 + b].broadcast_to([CHI, CLO]),
                                     phi[CHI:CLO, 2, b, :], start=False, stop=True)

                # W -> SBUF bf16 with the +K constant as a bias AP.
                wsb = wsbp.tile([CLO, 2, L], BF16, tag="wsb")
                nc.scalar.activation(wsb[:], ps_w[:, :, 0:L], ident, bias=kv)

                # combine: og = x*Wx + y*Wy over [chunk, b, n].  t1 and
                # t2-lo on DVE, t2-hi on gpsimd, final add on gpsimd.
                w_x = _reap(wsb[:], [[0, 2], [L, 2], [1, N]])
                w_y = _reap(wsb[:, :, N:L], [[0, 2], [L, 2], [1, N]])
                w_y_hi = _reap(wsb[0:CHI, :, N:L], [[L, 2], [1, N]])
                t1 = work.tile([CLO, 2, 2, N], BF16, tag="t1")
                t2 = work.tile([CLO, 2, 2, N], BF16, tag="t2")
                nc.vector.tensor_tensor(t1[:], xgb[:, :, b0:b0 + 2, 0, :], w_x, mult)
                nc.vector.tensor_tensor(t2[:, 0, :, :], xgb[:, 0, b0:b0 + 2, 1, :],
                                        _reap(wsb[:, :, N:L], [[L, 2], [1, N]]), mult)
                nc.gpsimd.tensor_tensor(t2[0:CHI, 1, :, :],
                                        xgb[0:CHI, 1, b0:b0 + 2, 1, :], w_y_hi, mult)
                nc.gpsimd.tensor_tensor(og[:, :, b0:b0 + 2, :], t1[:], t2[:], add)
                # HAM keep-warm fillers: PE activity drives the chip-wide
                # DVFS state.  Narrow 120-col bursts into unused psw cols.
                fill = psw.tile([CLO, 2, 512], F32, tag="ps_w")
                for s in (0, 1, 0, 1, 0, 1):
                    nc.tensor.matmul(fill[:, s, 392:512], wseed[:, 0:128],
                                     wseed[:, 0:120], start=True, stop=True)
                if u == GP - 1:
                    gb = 2 * GP * (p // GP)
                    nc.sync.dma_start(out=outt[0:CLO, gb:gb + 2 * GP, :],
                                      in_=og[:, 0, :, :])
                    nc.sync.dma_start(out=outt[CLO:C, gb:gb + 2 * GP, :],
                                      in_=og[0:CHI, 1, :, :])

            LAG = 2
            pending = []

            for g in range(NGRP):
                gb = 2 * GP * g
                xgb = xin.tile([CLO, 2, 2 * GP, 2, N], BF16, tag="xgb")
                nc.sync.dma_start(out=xgb[:, 0, :, :, :],
                                  in_=xyb[0:CLO, gb:gb + 2 * GP, :, :])
                nc.sync.dma_start(out=xgb[0:CHIA, 1, :, :, :],
                                  in_=xyb[CLO:C + 1, gb:gb + 2 * GP, :, :])
                og = outp.tile([CLO, 2, 2 * GP, N], BF16, tag="og")
                for u in range(GP):
                    if len(pending) >= LAG:
                        emit_back(pending.pop(0))
                    pending.append(emit_front(GP * g + u, xgb, og))
            for st in pending:
                emit_back(st)

    nc.compile()
    return nc


def _host_prepack(d):
    """Fold BN, collapse the linear tail, build constant blobs."""
    f = np.float32
    bf = ml_dtypes.bfloat16
    inv1 = d["g1"] / np.sqrt(d["v1"] + 1e-5)
    W1 = (d["w1"] * inv1[:, None]).astype(f)
    c1 = ((d["b1"] - d["m1"]) * inv1 + d["be1"]).astype(f)
    inv2 = d["g2"] / np.sqrt(d["v2"] + 1e-5)
    W2 = (d["w2"] * inv2[:, None]).astype(f)
    c2 = ((d["b2"] - d["m2"]) * inv2 + d["be2"]).astype(f)

    w4eff = d["w4"][:, :L] + d["w4"][:, L:]
    w5a, w5b = d["w5"][0, :C], d["w5"][0, C:]
    u3 = (w5a @ d["w3"]).astype(f)
    u4 = (w5b @ w4eff).astype(f)
    K = float(w5a @ d["b3"] + w5b @ d["b4"] + d["b5"][0])

    W1T = np.ascontiguousarray(W1.T)
    W2T = np.ascontiguousarray(W2.T)

    wblob = np.zeros((CLO, 768), bf)

    def pack_stream(c0, WT, cb):
        # A [128,128]: lo-contraction -> lo-out
        wblob[:, c0:c0 + 128] = WT[0:128, 0:128].astype(bf)
        # B [128,64]: lo-contraction -> hi-out
        wblob[:, c0 + 128:c0 + 192] = WT[0:128, 128:192].astype(bf)
        # C [65,128]: rows 0:64 = hi-contraction -> lo-out, row 64 = bias
        wblob[0:CHI, c0 + 192:c0 + 320] = WT[128:192, 0:128].astype(bf)
        wblob[CHI, c0 + 192:c0 + 320] = cb[0:128].astype(bf)
        # D [65,64]
        wblob[0:CHI, c0 + 320:c0 + 384] = WT[128:192, 128:192].astype(bf)
        wblob[CHI, c0 + 320:c0 + 384] = cb[128:192].astype(bf)

    pack_stream(0, W1T, c1)
    pack_stream(384, W2T, c2)

    # u4b: cols 0:392 = u4 (cat order) on all partitions; cols 392:588 =
    # hi-bank map (parts 0:64 -> u4a, parts 64:128 -> u4b).
    u4b = np.zeros((CLO, 588), bf)
    u4b[:, 0:L] = np.broadcast_to(u4.astype(bf), (CLO, L))
    u4b[0:CHI, L:588] = np.broadcast_to(u4[0:N].astype(bf), (CHI, N))
    u4b[CHI:CLO, L:588] = np.broadcast_to(u4[N:L].astype(bf), (CHI, N))

    u34 = np.zeros((CLO, 5), f)
    u34[:, 0] = u34[:, 1] = u3[0:128]
    u34[0:CHI, 2] = u3[128:192]      # u3-hi for the parts-0:64 lhsT copy
    u34[CHI:CLO, 3] = u3[128:192]    # u3-hi for the parts-64:128 lhsT copy
    u34[:, 4] = K
    foldm = (np.eye(CLO) + np.roll(np.eye(CLO), 64, axis=0)).astype(f)
    return {"wblob": wblob, "u4b": u4b, "u34": u34, "foldm": foldm}


def run(inputs, trace=False):
    d = {k: np.asarray(v) for k, v in inputs.items()}
    consts = _host_prepack(d)
    bf = ml_dtypes.bfloat16

    xyb = np.ones((C + 1, B, 2, N), bf)
    xyb[0:C, :, 0, :] = d["x"].transpose(2, 0, 1).astype(bf)
    xyb[0:C, :, 1, :] = d["y"].transpose(2, 0, 1).astype(bf)

    if "nc" not in _CACHE:
        _CACHE["nc"] = _build_program()
    nc = _CACHE["nc"]

    in_maps = []
    for cid in range(NCORES):
        m = dict(consts)
        m["xyb"] = np.ascontiguousarray(xyb[:, cid * NB:(cid + 1) * NB])
        in_maps.append(m)

    res = run_bass_kernel_spmd(nc, in_maps, list(range(NCORES)), trace=trace)
    # out HBM is [C, NB, N] bf16 channel-major; reassemble [B, C, N] f32
    out = np.concatenate(
        [res.results[i]["out"].astype(np.float32) for i in range(NCORES)], axis=1
    ).transpose(1, 0, 2)
    return np.ascontiguousarray(out), res


def kernel(**inputs):
    out, _ = run(inputs, trace=False)
    return out
